# revision 59
# baseline (speedup 1.0000x reference)
"""Multi-head cross-attention Trainium2 kernel (8 NeuronCores).

Problem shapes (hardcoded): query (4,512,256); key_value (4,256,64,64);
Wq/Wk/Wv/Wo (256,256); biases (256,). NUM_HEADS=8, HEAD_DIM=32.

Sharding: 8 cores = 4 batches x 2 head-groups (4 heads / 128 dims each).
Each core computes its head-group's attention for one batch plus the
partial output projection over its 128 contraction dims; the host adds
the two partials per batch plus (bv @ Wo.T + bo), which supplies exactly
the missing bias terms (softmax is invariant to bk; bv passes through the
attention weights unchanged).  Host-side sharding also lays the operands
out for the PE: q and the four weight matrices are pre-transposed and
pre-cast to fp16, kv is pre-cast to fp16, so the device spends no time on
layout work.

Per-core dataflow (S^T layout: kv position j on partitions, s on free; all
PE inputs fp16, PSUM accumulation fp32):
  K^T[dk,j]  = WkT.T @ kv          (PE)
  V[j,dv]    = kv.T @ WvT          (PE), packed as [V_h | ones] per head
  S^T[j,s]   = KT_h.T @ QT_h       (PE, K=32 row-tiled, 4 heads concurrent)
  P^T        = exp(scale*S^T)      (ACT, PSUM->SBUF fp16; the bottleneck:
               64 x [128,1024] exps ~= 65us, everything else hides under)
  att_cb    += [V|1].T @ P^T       (PE, M=64 col-tiled pairs, PSUM-acc;
               one PSUM bank per head pair: rows 0-31 out even, 32-63
               sums even, 64-95 out odd, 96-127 sums odd)
  attn^T     = out^T / sum         (DVE reciprocal + mul per head pair)
  out[s,do]  = attn^T.T @ WoT      (PE) --DMA--> DRAM
Softmax max-subtraction is skipped: scores are ~N(0,1) after the 1/sqrt(32)
scale, so exp() stays well inside fp32/fp16 range; results match
jax.nn.softmax up to fp rounding.

Scheduling notes:
  - DMA triggers cost ~650ns of engine time and GpSimd-issued transfers
    are PIO (they stall behind GpSimd compute), so loads are spread over
    the Sync and Scalar queues (Scalar is idle until the first exp).
  - kv streams in jc-PAIR loads ([128, 1024] fp16 per d-half): few, fat
    descriptors; pair 0's halves ride two different queues.
  - K^T/V projections and the Wo load are emitted BETWEEN waves: the PE
    executes its queue in order, so the filler work spreads across the
    whole stream and keeps the PE duty cycle high; that matters because
    the core hardware-throttles PE activity to ~50% after ~35us and PE
    slack is what absorbs it.
  - The att accumulator is two single-bank PSUM tiles (one head pair
    each) so the tail normalization of pair 0 cannot serialize against
    the final attnV of pair 1.
"""

import numpy as np

B, S, D = 4, 512, 256
HW = 4096
HD = 32  # head dim
DC = 128  # head-group width in D
N_CORES = 8
SCALE = float(HD) ** -0.5

_PROG_CACHE = {}


def _build_program():
    from contextlib import ExitStack

    import concourse.bass as bass  # noqa: F401
    import concourse.tile as tile
    from concourse import bacc, mybir

    f32 = mybir.dt.float32
    fp16 = mybir.dt.float16
    AF = mybir.ActivationFunctionType

    nc = bacc.Bacc("TRN2", target_bir_lowering=False, debug=False)

    # qT: [d-chunk c | p, (c, s)] so QT = WqT.T @ qT needs no on-device
    # transposes; weights likewise pre-transposed [d, dc] / [dc, do]
    qt_d = nc.dram_tensor("qT", [128, 1024], fp16, kind="ExternalInput").ap()
    kv_d = nc.dram_tensor("kv", [D, HW], fp16, kind="ExternalInput").ap()
    wqt_d = nc.dram_tensor("wqT", [128, 256], fp16, kind="ExternalInput").ap()
    wkt_d = nc.dram_tensor("wkT", [128, 256], fp16, kind="ExternalInput").ap()
    wvt_d = nc.dram_tensor("wvT", [128, 256], fp16, kind="ExternalInput").ap()
    wot_d = nc.dram_tensor("woT", [128, 256], fp16, kind="ExternalInput").ap()
    bq_d = nc.dram_tensor("bq", [DC], f32, kind="ExternalInput").ap()
    id_d = nc.dram_tensor("ident", [128, 128], fp16, kind="ExternalInput").ap()
    out_d = nc.dram_tensor("out", [S, D], f32, kind="ExternalOutput").ap()

    with tile.TileContext(nc, pool_alloc_mode="queue") as tc, ExitStack() as ctx:
        const_pool = ctx.enter_context(tc.tile_pool(name="const", bufs=1))
        wpool = ctx.enter_context(tc.tile_pool(name="wts", bufs=1))
        qpool = ctx.enter_context(tc.tile_pool(name="qstage", bufs=1))
        kvpool = ctx.enter_context(tc.tile_pool(name="kv", bufs=6))
        ktpool = ctx.enter_context(tc.tile_pool(name="kt", bufs=4))
        ptpool = ctx.enter_context(tc.tile_pool(name="pt", bufs=6))
        mpool = ctx.enter_context(tc.tile_pool(name="misc", bufs=1))
        # PSUM: 2x[128,1024] score slots (4 banks) + 2x[128,512] proj slots
        # (2 banks) + 2x[128,512] att accumulators (2 banks) = 8 banks
        ps_work = ctx.enter_context(tc.tile_pool(name="psw", bufs=2, space="PSUM"))
        ps_kv = ctx.enter_context(tc.tile_pool(name="pskv", bufs=2, space="PSUM"))
        ps_att = ctx.enter_context(tc.tile_pool(name="psa", bufs=1, space="PSUM"))

        # ---- prologue DMAs (DMA issue ~650ns of engine time each) ----
        # kv loads in jc-PAIRS ([128, 1024] fp16 per d-half, 2KB runs); the
        # two halves of pair 0 go on DIFFERENT queues so they don't
        # serialize ahead of the first K^T.
        # Scalar (idle until the first exp): wkT, kv0 half a, wvT.
        warm_in = const_pool.tile([128, 1], f32, tag="warm_in")
        nc.gpsimd.memset(warm_in[:], 0.0)
        wkT = wpool.tile([128, 256], fp16, tag="wkT")
        nc.scalar.dma_start(wkT[:], wkt_d[:, :])
        bq_sb = wpool.tile([128, 1], f32, tag="bq")
        nc.scalar.dma_start(bq_sb[:], bq_d.unsqueeze(1))
        kv0 = kvpool.tile([128, 1024], fp16, tag="kv", name="kv0")
        nc.scalar.dma_start(kv0[:], kv_d[0:128, 0:1024])
        wvT = wpool.tile([128, 256], fp16, tag="wvT")
        nc.scalar.dma_start(wvT[:], wvt_d[:, :])
        # Sync: qT + wqT (the QT chain), kv0 half b, ident
        qT = qpool.tile([128, 1024], fp16, tag="qT")
        nc.sync.dma_start(qT[:], qt_d[:, :])
        wqT = wpool.tile([128, 256], fp16, tag="wqT")
        nc.sync.dma_start(wqT[:], wqt_d[:, :])
        kv1 = kvpool.tile([128, 1024], fp16, tag="kv", name="kv1")
        nc.sync.dma_start(kv1[:], kv_d[128:256, 0:1024])
        ident = const_pool.tile([128, 128], fp16)
        nc.sync.dma_start(ident[:], id_d[:, :])
        # warmup exp AFTER the scalar DMA issues: the hoisted ACT table load
        # then runs behind them instead of delaying kv half a
        warm_out = const_pool.tile([128, 1], f32, tag="warm_out")
        nc.scalar.activation(warm_out[:], warm_in[:], AF.Exp)

        kvs = {0: (kv0, kv1)}  # keyed by jc PAIR (jc//2)
        kts = {}

        def kv_slices(jc):
            off = 512 * (jc % 2)
            pair = kvs[jc // 2]
            return pair[0][:, off : off + 512], pair[1][:, off : off + 512]

        def emit_ktproj(jc):
            kha, khb = kv_slices(jc)
            kt_ps = ps_kv.tile([128, 512], f32, tag="kvp", name="kt_ps")
            nc.tensor.matmul(kt_ps[:], wkT[:, 0:128], kha, start=True, stop=False)
            nc.tensor.matmul(kt_ps[:], wkT[:, 128:256], khb, start=False, stop=True)
            kt_sb = ktpool.tile([128, 512], fp16, tag="kt", name="kt_sb")
            nc.vector.tensor_copy(kt_sb[:], kt_ps[:])
            kts[jc] = kt_sb

        # ---- prologue chain to the first exp: QT matmuls + jc0 K^T ----
        qt_ps = ps_work.tile([128, 512], f32, tag="w", name="qt_ps")
        for c in range(2):
            nc.tensor.matmul(
                qt_ps[:],
                wqT[:, 128 * c : 128 * (c + 1)],
                qT[:, 512 * c : 512 * (c + 1)],
                start=(c == 0),
                stop=(c == 1),
            )
        emit_ktproj(0)
        QT = qpool.tile([128, 512], fp16, tag="QT")
        nc.vector.tensor_scalar_add(QT[:], qt_ps[:], bq_sb[:])

        # persistent V|ones ring: 8 slots of [128, 1024]; slot cols per
        # (js, h): [64*h : +32] = V_h, [+32 : +64] = ones.  All ones set
        # once here on GpSimd (no GpSimd DMAs exist to stall, and the DVE
        # stays free for the prologue casts).
        v_ring = const_pool.tile([128, 8192], fp16, tag="vring")
        for hreg in range(2):
            nc.gpsimd.memset(
                v_ring[:, 4096 * hreg : 4096 * (hreg + 1)]
                .rearrange("p (rj he x) -> p rj he x", rj=16, he=8, x=32)[
                    :, :, 1::2, :
                ],
                1.0,
            )

        # att accumulators: one PSUM bank per head pair (cb); rows
        # 0-31 out^T_{2cb}, 32-63 sums_{2cb}, 64-95 out^T_{2cb+1},
        # 96-127 sums_{2cb+1} (32-aligned so DVE ops can read PSUM directly)
        att = [
            ps_att.tile([128, 512], f32, tag=f"att{cb}", name=f"att{cb}")
            for cb in range(2)
        ]

        wo_state = {}

        def emit_wo_prep(sub):
            if sub == 0:
                woT = wpool.tile([128, 256], fp16, tag="woT", name="woT")
                nc.sync.dma_start(woT[:], wot_d[:, :])
                wo_state["woT"] = woT
            else:
                wo_state["attn"] = mpool.tile(
                    [128, 512], fp16, tag="attn", name="attn"
                )

        def emit_tail_cb(cb):
            """Normalize head pair cb.  The sum gathers run as Copy
            activations on the Scalar engine (idle after the last exp; Copy
            shares Exp's act table), so the DVE only runs one reciprocal and
            two scaling muls per pair."""
            a = att[cb]
            rs = mpool.tile([64, 512], f32, tag=f"rs{cb}", name=f"rs{cb}")
            nc.scalar.activation(rs[0:32, :], a[32:64, :], AF.Copy)
            nc.scalar.activation(rs[32:64, :], a[96:128, :], AF.Copy)
            rsum = mpool.tile([64, 512], f32, tag=f"rsum{cb}", name=f"rsum{cb}")
            nc.vector.reciprocal_approx_fast(rsum[:], rs[:])
            attn = wo_state["attn"]
            h0 = 2 * cb
            nc.vector.tensor_mul(
                attn[32 * h0 : 32 * h0 + 32, :], a[0:32, :], rsum[0:32, :]
            )
            nc.vector.tensor_mul(
                attn[32 * h0 + 32 : 32 * h0 + 64, :], a[64:96, :], rsum[32:64, :]
            )

        def emit_kv_dma(pair):
            kva = kvpool.tile([128, 1024], fp16, tag="kv", name="kva")
            kvb = kvpool.tile([128, 1024], fp16, tag="kv", name="kvb")
            nc.sync.dma_start(kva[:], kv_d[0:128, 1024 * pair : 1024 * (pair + 1)])
            nc.sync.dma_start(kvb[:], kv_d[128:256, 1024 * pair : 1024 * (pair + 1)])
            kvs[pair] = (kva, kvb)

        def emit_v(jc):
            """V projection for block jc into v_ring slot jc (+ its ones).
            Computed as V^T (two full-width matmuls) then PE-transposed:
            fewer, fatter PE instructions than eight N=128 matmuls."""
            kha, khb = kv_slices(jc)
            vt_ps = ps_kv.tile([128, 512], f32, tag="kvp", name="vt_ps")
            nc.tensor.matmul(vt_ps[:], wvT[:, 0:128], kha, start=True, stop=False)
            nc.tensor.matmul(vt_ps[:], wvT[:, 128:256], khb, start=False, stop=True)
            vt_sb = ktpool.tile([128, 512], fp16, tag="vt", name="vt_sb")
            nc.vector.tensor_copy(vt_sb[:], vt_ps[:])
            vtr_ps = ps_kv.tile([128, 512], fp16, tag="kvp", name="vtr_ps")
            for t in range(4):
                nc.tensor.transpose(
                    vtr_ps[:, 128 * t : 128 * (t + 1)],
                    vt_sb[:, 128 * t : 128 * (t + 1)],
                    ident[:],
                )
            v_sb = v_ring[:, 1024 * jc : 1024 * (jc + 1)]
            v_dst = v_sb.rearrange("p (js he x) -> p js he x", js=4, he=8, x=32)
            for js in range(4):
                nc.vector.tensor_copy(
                    # even he positions are the V columns
                    v_dst[:, js, 0::2, :],
                    vtr_ps[:, 128 * js : 128 * (js + 1)].rearrange(
                        "p (h x) -> p h x", x=32
                    ),
                )

        # ---- main streaming loop: 32 waves of 128 kv positions ----
        emit_kv_dma(1)

        for jc in range(8):  # 512-wide kv blocks
            kt_sb = kts.pop(jc)
            v_sb = v_ring[:, 1024 * jc : 1024 * (jc + 1)]
            for js in range(4):  # 128-wide j waves
                first = jc == 0 and js == 0
                last = jc == 7 and js == 3
                sc_a = ps_work.tile([128, 1024], f32, tag="w", name="sc_a")
                sc_b = ps_work.tile([128, 1024], f32, tag="w", name="sc_b")
                scs = [sc_a, sc_b]
                for h in range(4):
                    nc.tensor.matmul(
                        scs[h // 2][:, 512 * (h % 2) : 512 * (h % 2) + 512],
                        kt_sb[32 * h : 32 * (h + 1), 128 * js : 128 * (js + 1)],
                        QT[32 * h : 32 * (h + 1), :],
                        start=True,
                        stop=True,
                        tile_position=(32 * h, 0),
                    )
                pts = []
                for hp in range(2):
                    pt = ptpool.tile([128, 1024], fp16, tag="pt", name="pt")
                    nc.scalar.activation(pt[:], scs[hp][:], AF.Exp, scale=SCALE)
                    pts.append(pt)

                # interleave points: queued on the PE between this wave's
                # scores and attnV so the projection work spreads across the
                # stream (PE executes in order)
                if js == 0 and jc == 0:
                    emit_v(0)
                if js == 1 and jc in (0, 2, 4):
                    emit_kv_dma(jc // 2 + 1)
                if js == 2 and jc < 7:
                    emit_ktproj(jc + 1)
                if js == 3 and jc < 7:
                    emit_v(jc + 1)
                if jc == 6 and js in (0, 3):
                    emit_wo_prep(0 if js == 0 else 1)

                for hp in range(2):
                    pt = pts[hp]
                    for hh in range(2):
                        h = 2 * hp + hh
                        nc.tensor.matmul(
                            att[hp][64 * hh : 64 * hh + 64, :],
                            v_sb[:, 256 * js + 64 * h : 256 * js + 64 * (h + 1)],
                            pt[:, 512 * hh : 512 * (hh + 1)],
                            start=first,
                            stop=last,
                            tile_position=(0, 64 * hh),
                            # the two head groups touch disjoint partition
                            # ranges of the bank; the lint is partition-unaware
                            skip_group_check=True,
                        )

        # ---- tail: normalize both head pairs, then project per s-chunk.
        # The projection contraction is split by head pair (row-tiled K=64
        # halves) so the first half runs while pair 1 is still normalizing.
        emit_tail_cb(0)
        emit_tail_cb(1)
        woT = wo_state["woT"]
        attn = wo_state["attn"]
        o_sb = mpool.tile([128, 1024], f32, tag="osb")
        for sc in range(4):
            o_ps = ps_work.tile([128, 1024], f32, tag="w", name="o_ps")
            nc.tensor.matmul(
                o_ps[:, 0:256],
                attn[:, 128 * sc : 128 * (sc + 1)],
                woT[:],
                start=True,
                stop=True,
            )
            nc.vector.tensor_copy(o_sb[:, 256 * sc : 256 * (sc + 1)], o_ps[:, 0:256])
            nc.sync.dma_start(
                out_d[128 * sc : 128 * (sc + 1), :],
                o_sb[:, 256 * sc : 256 * (sc + 1)],
            )

    nc.compile()
    return nc


def get_program():
    if "nc" not in _PROG_CACHE:
        _PROG_CACHE["nc"] = _build_program()
    return _PROG_CACHE["nc"]


def make_in_maps(query, key_value, Wq, bq, Wk, bk, Wv, bv, Wo, bo):
    query = np.asarray(query, dtype=np.float32)
    key_value = np.asarray(key_value, dtype=np.float32)
    Wq = np.asarray(Wq, dtype=np.float32)
    Wk = np.asarray(Wk, dtype=np.float32)
    Wv = np.asarray(Wv, dtype=np.float32)
    Wo = np.asarray(Wo, dtype=np.float32)
    bq = np.asarray(bq, dtype=np.float32)

    def wt(Wsl):  # [128, 256]: wt[p, 128c+m] = Wsl[m, 128c+p]
        return np.ascontiguousarray(
            np.concatenate([Wsl[:, 0:128].T, Wsl[:, 128:256].T], axis=1)
        ).astype(np.float16)

    in_maps = []
    for c in range(N_CORES):
        b, g = c // 2, c % 2
        sl = slice(g * DC, (g + 1) * DC)
        qb = query[b]
        qT = np.ascontiguousarray(
            np.concatenate([qb[:, 0:128].T, qb[:, 128:256].T], axis=1)
        ).astype(np.float16)
        in_maps.append(
            {
                "qT": qT,
                "kv": key_value[b].reshape(D, HW).astype(np.float16),
                "wqT": wt(Wq[sl]),
                "wkT": wt(Wk[sl]),
                "wvT": wt(Wv[sl]),
                "woT": np.ascontiguousarray(Wo[:, sl].T).astype(np.float16),
                "bq": np.ascontiguousarray(bq[sl]),
                "ident": np.eye(128, dtype=np.float16),
            }
        )
    return in_maps


def run_on_cores(in_maps, trace=False):
    from concourse import bass_utils

    nc = get_program()
    return bass_utils.run_bass_kernel_spmd(
        nc, in_maps, core_ids=list(range(N_CORES)), trace=trace
    )


def kernel(query, key_value, Wq, bq, Wk, bk, Wv, bv, Wo, bo):
    in_maps = make_in_maps(query, key_value, Wq, bq, Wk, bk, Wv, bv, Wo, bo)
    res = run_on_cores(in_maps)
    Wo_np = np.asarray(Wo, dtype=np.float32)
    bias = np.asarray(bv, dtype=np.float32) @ Wo_np.T + np.asarray(
        bo, dtype=np.float32
    )
    out = np.empty((B, S, D), dtype=np.float32)
    for b in range(B):
        out[b] = res.results[2 * b]["out"] + res.results[2 * b + 1]["out"] + bias
    return out


# revision 60
# speedup vs baseline: 1.2075x; 1.2075x over previous
"""Multi-head cross-attention Trainium2 kernel (8 NeuronCores).

Problem shapes (hardcoded): query (4,512,256); key_value (4,256,64,64);
Wq/Wk/Wv/Wo (256,256); biases (256,). NUM_HEADS=8, HEAD_DIM=32.

Sharding: 8 cores = 4 batches x 2 head-groups (4 heads / 128 dims each).
Each core computes its head-group's attention for one batch plus the
partial output projection over its 128 contraction dims; the host adds
the two partials per batch plus (bv @ Wo.T + bo), which supplies exactly
the missing bias terms (softmax is invariant to bk; bv passes through the
attention weights unchanged).  Host-side sharding also lays the operands
out for the PE: q and the four weight matrices are pre-transposed and
pre-cast to fp16, kv is pre-cast to fp16, so the device spends no time on
layout work.

Per-core dataflow (S^T layout: kv position j on partitions, s on free; all
PE inputs fp16, PSUM accumulation fp32):
  K^T[dk,j]  = WkT.T @ kv          (PE)
  V[j,dv]    = kv.T @ WvT          (PE), packed as [V_h | ones] per head
  S^T[j,s]   = KT_h.T @ QT_h       (PE, K=32 row-tiled, 4 heads concurrent)
  P^T        = exp(scale*S^T)      (ACT, PSUM->SBUF fp16; the bottleneck:
               64 x [128,1024] exps ~= 65us, everything else hides under)
  att_cb    += [V|1].T @ P^T       (PE, M=64 col-tiled pairs, PSUM-acc;
               one PSUM bank per head pair: rows 0-31 out even, 32-63
               sums even, 64-95 out odd, 96-127 sums odd)
  attn^T     = out^T / sum         (DVE reciprocal + mul per head pair)
  out[s,do]  = attn^T.T @ WoT      (PE) --DMA--> DRAM
Softmax max-subtraction is skipped: scores are ~N(0,1) after the 1/sqrt(32)
scale, so exp() stays well inside fp32/fp16 range; results match
jax.nn.softmax up to fp rounding.

Scheduling notes:
  - DMA triggers cost ~650ns of engine time and GpSimd-issued transfers
    are PIO (they stall behind GpSimd compute), so loads are spread over
    the Sync and Scalar queues (Scalar is idle until the first exp).
  - kv streams in jc-PAIR loads ([128, 1024] fp16 per d-half): few, fat
    descriptors; pair 0's halves ride two different queues.
  - K^T/V projections and the Wo load are emitted BETWEEN waves: the PE
    executes its queue in order, so the filler work spreads across the
    whole stream and keeps the PE duty cycle high; that matters because
    the core hardware-throttles PE activity to ~50% after ~35us and PE
    slack is what absorbs it.
  - The att accumulator is two single-bank PSUM tiles (one head pair
    each) so the tail normalization of pair 0 cannot serialize against
    the final attnV of pair 1.
"""

import numpy as np

B, S, D = 4, 512, 256
HW = 4096
HD = 32  # head dim
DC = 128  # head-group width in D
N_CORES = 8
SCALE = float(HD) ** -0.5

_PROG_CACHE = {}


def _build_program():
    from contextlib import ExitStack

    import concourse.bass as bass  # noqa: F401
    import concourse.tile as tile
    from concourse import bacc, mybir

    f32 = mybir.dt.float32
    fp16 = mybir.dt.float16
    AF = mybir.ActivationFunctionType

    nc = bacc.Bacc("TRN2", target_bir_lowering=False, debug=False)

    # qT: [d-chunk c | p, (c, s)] so QT = WqT.T @ qT needs no on-device
    # transposes; weights likewise pre-transposed [d, dc] / [dc, do]
    qt_d = nc.dram_tensor("qT", [128, 1024], fp16, kind="ExternalInput").ap()
    kv_d = nc.dram_tensor("kv", [D, HW], fp16, kind="ExternalInput").ap()
    wqt_d = nc.dram_tensor("wqT", [128, 256], fp16, kind="ExternalInput").ap()
    wkt_d = nc.dram_tensor("wkT", [128, 256], fp16, kind="ExternalInput").ap()
    wvt_d = nc.dram_tensor("wvT", [128, 256], fp16, kind="ExternalInput").ap()
    wot_d = nc.dram_tensor("woT", [128, 256], fp16, kind="ExternalInput").ap()
    bq_d = nc.dram_tensor("bq", [DC], f32, kind="ExternalInput").ap()
    id_d = nc.dram_tensor("ident", [128, 128], fp16, kind="ExternalInput").ap()
    out_d = nc.dram_tensor("out", [S, D], f32, kind="ExternalOutput").ap()

    with tile.TileContext(nc, pool_alloc_mode="queue") as tc, ExitStack() as ctx:
        const_pool = ctx.enter_context(tc.tile_pool(name="const", bufs=1))
        wpool = ctx.enter_context(tc.tile_pool(name="wts", bufs=1))
        qpool = ctx.enter_context(tc.tile_pool(name="qstage", bufs=1))
        kvpool = ctx.enter_context(tc.tile_pool(name="kv", bufs=6))
        ktpool = ctx.enter_context(tc.tile_pool(name="kt", bufs=4))
        ptpool = ctx.enter_context(tc.tile_pool(name="pt", bufs=6))
        mpool = ctx.enter_context(tc.tile_pool(name="misc", bufs=1))
        # PSUM: 2x[128,1024] score slots (4 banks) + 2x[128,512] proj slots
        # (2 banks) + 2x[128,512] att accumulators (2 banks) = 8 banks
        ps_work = ctx.enter_context(tc.tile_pool(name="psw", bufs=2, space="PSUM"))
        ps_kv = ctx.enter_context(tc.tile_pool(name="pskv", bufs=2, space="PSUM"))
        ps_att = ctx.enter_context(tc.tile_pool(name="psa", bufs=1, space="PSUM"))

        # ---- prologue DMAs (DMA issue ~650ns of engine time each) ----
        # kv loads in jc-PAIRS ([128, 1024] fp16 per d-half, 2KB runs); the
        # two halves of pair 0 go on DIFFERENT queues so they don't
        # serialize ahead of the first K^T.
        # Scalar (idle until the first exp): wkT, kv0 half a, wvT.
        warm_in = const_pool.tile([128, 1], f32, tag="warm_in")
        nc.gpsimd.memset(warm_in[:], 0.0)
        # The K^T chain (kv pair 0 -> K^T -> kt cast -> scores) is the
        # longest pole to the first exp: both kv halves LEAD their queues.
        kv0 = kvpool.tile([128, 1024], fp16, tag="kv", name="kv0")
        nc.scalar.dma_start(kv0[:], kv_d[0:128, 0:1024])
        wkT = wpool.tile([128, 256], fp16, tag="wkT")
        nc.scalar.dma_start(wkT[:], wkt_d[:, :])
        bq_sb = wpool.tile([128, 1], f32, tag="bq")
        nc.scalar.dma_start(bq_sb[:], bq_d.unsqueeze(1))
        wvT = wpool.tile([128, 256], fp16, tag="wvT")
        nc.scalar.dma_start(wvT[:], wvt_d[:, :])
        # Sync: kv0 half b first, then the QT chain (qT + wqT), ident
        kv1 = kvpool.tile([128, 1024], fp16, tag="kv", name="kv1")
        nc.sync.dma_start(kv1[:], kv_d[128:256, 0:1024])
        qT = qpool.tile([128, 1024], fp16, tag="qT")
        nc.sync.dma_start(qT[:], qt_d[:, :])
        wqT = wpool.tile([128, 256], fp16, tag="wqT")
        nc.sync.dma_start(wqT[:], wqt_d[:, :])
        ident = const_pool.tile([128, 128], fp16)
        nc.sync.dma_start(ident[:], id_d[:, :])
        # warmup exp AFTER the scalar DMA issues: the hoisted ACT table load
        # then runs behind them instead of delaying kv half a
        warm_out = const_pool.tile([128, 1], f32, tag="warm_out")
        nc.scalar.activation(warm_out[:], warm_in[:], AF.Exp)

        kvs = {0: (kv0, kv1)}  # keyed by jc PAIR (jc//2)
        kts = {}

        def kv_slices(jc):
            off = 512 * (jc % 2)
            pair = kvs[jc // 2]
            return pair[0][:, off : off + 512], pair[1][:, off : off + 512]

        def emit_ktproj(jc):
            kha, khb = kv_slices(jc)
            kt_ps = ps_kv.tile([128, 512], f32, tag="kvp", name="kt_ps")
            nc.tensor.matmul(kt_ps[:], wkT[:, 0:128], kha, start=True, stop=False)
            nc.tensor.matmul(kt_ps[:], wkT[:, 128:256], khb, start=False, stop=True)
            kt_sb = ktpool.tile([128, 512], fp16, tag="kt", name="kt_sb")
            nc.vector.tensor_copy(kt_sb[:], kt_ps[:])
            kts[jc] = kt_sb

        # ---- prologue chain to the first exp: QT matmuls + jc0 K^T ----
        qt_ps = ps_work.tile([128, 512], f32, tag="w", name="qt_ps")
        for c in range(2):
            nc.tensor.matmul(
                qt_ps[:],
                wqT[:, 128 * c : 128 * (c + 1)],
                qT[:, 512 * c : 512 * (c + 1)],
                start=(c == 0),
                stop=(c == 1),
            )
        emit_ktproj(0)
        QT = qpool.tile([128, 512], fp16, tag="QT")
        nc.vector.tensor_scalar_add(QT[:], qt_ps[:], bq_sb[:])

        # persistent V|ones ring: 8 slots of [128, 1024]; slot cols per
        # (js, h): [64*h : +32] = V_h, [+32 : +64] = ones.  All ones set
        # once here on GpSimd (no GpSimd DMAs exist to stall, and the DVE
        # stays free for the prologue casts).
        v_ring = const_pool.tile([128, 8192], fp16, tag="vring")
        for hreg in range(2):
            nc.gpsimd.memset(
                v_ring[:, 4096 * hreg : 4096 * (hreg + 1)]
                .rearrange("p (rj he x) -> p rj he x", rj=16, he=8, x=32)[
                    :, :, 1::2, :
                ],
                1.0,
            )

        # att accumulators: one PSUM bank per head pair (cb); rows
        # 0-31 out^T_{2cb}, 32-63 sums_{2cb}, 64-95 out^T_{2cb+1},
        # 96-127 sums_{2cb+1} (32-aligned so DVE ops can read PSUM directly)
        att = [
            ps_att.tile([128, 512], f32, tag=f"att{cb}", name=f"att{cb}")
            for cb in range(2)
        ]

        wo_state = {}

        def emit_wo_prep(sub):
            if sub == 0:
                woT = wpool.tile([128, 256], fp16, tag="woT", name="woT")
                nc.sync.dma_start(woT[:], wot_d[:, :])
                wo_state["woT"] = woT
            else:
                wo_state["attn"] = mpool.tile(
                    [128, 512], fp16, tag="attn", name="attn"
                )

        def emit_tail_cb(cb):
            """Normalize head pair cb.  The sum gathers run as Copy
            activations on the Scalar engine (idle after the last exp; Copy
            shares Exp's act table), so the DVE only runs one reciprocal and
            two scaling muls per pair."""
            a = att[cb]
            rs = mpool.tile([64, 512], f32, tag=f"rs{cb}", name=f"rs{cb}")
            nc.scalar.activation(rs[0:32, :], a[32:64, :], AF.Copy)
            nc.scalar.activation(rs[32:64, :], a[96:128, :], AF.Copy)
            rsum = mpool.tile([64, 512], f32, tag=f"rsum{cb}", name=f"rsum{cb}")
            nc.vector.reciprocal_approx_fast(rsum[:], rs[:])
            attn = wo_state["attn"]
            h0 = 2 * cb
            nc.vector.tensor_mul(
                attn[32 * h0 : 32 * h0 + 32, :], a[0:32, :], rsum[0:32, :]
            )
            nc.vector.tensor_mul(
                attn[32 * h0 + 32 : 32 * h0 + 64, :], a[64:96, :], rsum[32:64, :]
            )

        def emit_kv_dma(pair):
            kva = kvpool.tile([128, 1024], fp16, tag="kv", name="kva")
            kvb = kvpool.tile([128, 1024], fp16, tag="kv", name="kvb")
            nc.sync.dma_start(kva[:], kv_d[0:128, 1024 * pair : 1024 * (pair + 1)])
            nc.sync.dma_start(kvb[:], kv_d[128:256, 1024 * pair : 1024 * (pair + 1)])
            kvs[pair] = (kva, kvb)

        def emit_v(jc):
            """V projection for block jc into v_ring slot jc (+ its ones).
            Computed as V^T (two full-width matmuls) then PE-transposed:
            fewer, fatter PE instructions than eight N=128 matmuls."""
            kha, khb = kv_slices(jc)
            vt_ps = ps_kv.tile([128, 512], f32, tag="kvp", name="vt_ps")
            nc.tensor.matmul(vt_ps[:], wvT[:, 0:128], kha, start=True, stop=False)
            nc.tensor.matmul(vt_ps[:], wvT[:, 128:256], khb, start=False, stop=True)
            vt_sb = ktpool.tile([128, 512], fp16, tag="vt", name="vt_sb")
            nc.vector.tensor_copy(vt_sb[:], vt_ps[:])
            vtr_ps = ps_kv.tile([128, 512], fp16, tag="kvp", name="vtr_ps")
            for t in range(4):
                nc.tensor.transpose(
                    vtr_ps[:, 128 * t : 128 * (t + 1)],
                    vt_sb[:, 128 * t : 128 * (t + 1)],
                    ident[:],
                )
            v_sb = v_ring[:, 1024 * jc : 1024 * (jc + 1)]
            v_dst = v_sb.rearrange("p (js he x) -> p js he x", js=4, he=8, x=32)
            for js in range(4):
                nc.vector.tensor_copy(
                    # even he positions are the V columns
                    v_dst[:, js, 0::2, :],
                    vtr_ps[:, 128 * js : 128 * (js + 1)].rearrange(
                        "p (h x) -> p h x", x=32
                    ),
                )

        # ---- main streaming loop: 32 waves of 128 kv positions ----
        emit_kv_dma(1)

        for jc in range(8):  # 512-wide kv blocks
            kt_sb = kts.pop(jc)
            v_sb = v_ring[:, 1024 * jc : 1024 * (jc + 1)]
            for js in range(4):  # 128-wide j waves
                first = jc == 0 and js == 0
                last = jc == 7 and js == 3
                sc_a = ps_work.tile([128, 1024], f32, tag="w", name="sc_a")
                sc_b = ps_work.tile([128, 1024], f32, tag="w", name="sc_b")
                scs = [sc_a, sc_b]
                for h in range(4):
                    nc.tensor.matmul(
                        scs[h // 2][:, 512 * (h % 2) : 512 * (h % 2) + 512],
                        kt_sb[32 * h : 32 * (h + 1), 128 * js : 128 * (js + 1)],
                        QT[32 * h : 32 * (h + 1), :],
                        start=True,
                        stop=True,
                        tile_position=(32 * h, 0),
                    )
                pts = []
                for hp in range(2):
                    pt = ptpool.tile([128, 1024], fp16, tag="pt", name="pt")
                    nc.scalar.activation(pt[:], scs[hp][:], AF.Exp, scale=SCALE)
                    pts.append(pt)

                # interleave points: queued on the PE between this wave's
                # scores and attnV so the projection work spreads across the
                # stream (PE executes in order)
                if js == 0 and jc == 0:
                    emit_v(0)
                if js == 1 and jc in (0, 2, 4):
                    emit_kv_dma(jc // 2 + 1)
                if js == 2 and jc < 7:
                    emit_ktproj(jc + 1)
                if js == 3 and jc < 7:
                    emit_v(jc + 1)
                if jc == 6 and js in (0, 3):
                    emit_wo_prep(0 if js == 0 else 1)

                for hp in range(2):
                    pt = pts[hp]
                    for hh in range(2):
                        h = 2 * hp + hh
                        nc.tensor.matmul(
                            att[hp][64 * hh : 64 * hh + 64, :],
                            v_sb[:, 256 * js + 64 * h : 256 * js + 64 * (h + 1)],
                            pt[:, 512 * hh : 512 * (hh + 1)],
                            start=first,
                            stop=last,
                            tile_position=(0, 64 * hh),
                            # the two head groups touch disjoint partition
                            # ranges of the bank; the lint is partition-unaware
                            skip_group_check=True,
                        )

        # ---- tail: normalize both head pairs, then project per s-chunk.
        # The projection contraction is split by head pair (row-tiled K=64
        # halves) so the first half runs while pair 1 is still normalizing.
        emit_tail_cb(0)
        emit_tail_cb(1)
        woT = wo_state["woT"]
        attn = wo_state["attn"]
        o_sb = mpool.tile([128, 1024], f32, tag="osb")
        for sc in range(4):
            o_ps = ps_work.tile([128, 1024], f32, tag="w", name="o_ps")
            nc.tensor.matmul(
                o_ps[:, 0:256],
                attn[:, 128 * sc : 128 * (sc + 1)],
                woT[:],
                start=True,
                stop=True,
            )
            nc.vector.tensor_copy(o_sb[:, 256 * sc : 256 * (sc + 1)], o_ps[:, 0:256])
            nc.sync.dma_start(
                out_d[128 * sc : 128 * (sc + 1), :],
                o_sb[:, 256 * sc : 256 * (sc + 1)],
            )

    nc.compile()
    return nc


def get_program():
    if "nc" not in _PROG_CACHE:
        _PROG_CACHE["nc"] = _build_program()
    return _PROG_CACHE["nc"]


def make_in_maps(query, key_value, Wq, bq, Wk, bk, Wv, bv, Wo, bo):
    query = np.asarray(query, dtype=np.float32)
    key_value = np.asarray(key_value, dtype=np.float32)
    Wq = np.asarray(Wq, dtype=np.float32)
    Wk = np.asarray(Wk, dtype=np.float32)
    Wv = np.asarray(Wv, dtype=np.float32)
    Wo = np.asarray(Wo, dtype=np.float32)
    bq = np.asarray(bq, dtype=np.float32)

    def wt(Wsl):  # [128, 256]: wt[p, 128c+m] = Wsl[m, 128c+p]
        return np.ascontiguousarray(
            np.concatenate([Wsl[:, 0:128].T, Wsl[:, 128:256].T], axis=1)
        ).astype(np.float16)

    in_maps = []
    for c in range(N_CORES):
        b, g = c // 2, c % 2
        sl = slice(g * DC, (g + 1) * DC)
        qb = query[b]
        qT = np.ascontiguousarray(
            np.concatenate([qb[:, 0:128].T, qb[:, 128:256].T], axis=1)
        ).astype(np.float16)
        in_maps.append(
            {
                "qT": qT,
                "kv": key_value[b].reshape(D, HW).astype(np.float16),
                "wqT": wt(Wq[sl]),
                "wkT": wt(Wk[sl]),
                "wvT": wt(Wv[sl]),
                "woT": np.ascontiguousarray(Wo[:, sl].T).astype(np.float16),
                "bq": np.ascontiguousarray(bq[sl]),
                "ident": np.eye(128, dtype=np.float16),
            }
        )
    return in_maps


def run_on_cores(in_maps, trace=False):
    from concourse import bass_utils

    nc = get_program()
    return bass_utils.run_bass_kernel_spmd(
        nc, in_maps, core_ids=list(range(N_CORES)), trace=trace
    )


def kernel(query, key_value, Wq, bq, Wk, bk, Wv, bv, Wo, bo):
    in_maps = make_in_maps(query, key_value, Wq, bq, Wk, bk, Wv, bv, Wo, bo)
    res = run_on_cores(in_maps)
    Wo_np = np.asarray(Wo, dtype=np.float32)
    bias = np.asarray(bv, dtype=np.float32) @ Wo_np.T + np.asarray(
        bo, dtype=np.float32
    )
    out = np.empty((B, S, D), dtype=np.float32)
    for b in range(B):
        out[b] = res.results[2 * b]["out"] + res.results[2 * b + 1]["out"] + bias
    return out
